# revision 59
# baseline (speedup 1.0000x reference)
"""Causal multi-head attention (B=2, S=2048, D=1024, H=16) on 8 TRN2 NeuronCores.

Head-parallel (tensor-parallel) sharding: core c owns heads (2c, 2c+1) for
BOTH batches. The full x is fed to every core from host HBM (pre-transposed,
st-major-contiguous), so q/k/v for the core's heads are computed locally and
causal attention needs NO k/v collective at all (the v2 baseline spent ~170us
on 8 serialized AllGathers at ~45 GB/s mesh bandwidth).

Output rows are owned interleaved-by-batch (core c: batch-0 rows [256c,256c+256)
plus batch-1 rows [256c,256c+256)), so the head-split -> row-split ctx reshard
splits into TWO junk-free 512KB 8-way AllToAlls: batch 0's hides completely
under batch 1's attention; only batch 1's (~25us) is exposed, and the batch-0
half of the output projection runs while it is in flight.

Attention per (batch, 256-row q-tile j): kv blocks 0..2j+1 as job-pairs
(jp = 2 kv blocks x 2 heads), QK packed 2-heads-per-PE-pass via tile_position,
exp on ACT in alternating [128,2048]/[128,1024] chunks (A/B PSUM rings; ACT
cost is (N+352)/1.2GHz so bigger chunks amortize the fixed overhead), softmax
denominators via a ones-column in v (psc PSUM bank zero-initialized once per
q-tile because matmul start=True resets has_written for the whole 2KB bank),
per-q-tile normalization via reciprocal_approx_fast + E-matrix broadcast
matmul feeding the A2A staging as soon as each q-tile finishes. Only the
diagonal job-pair needs a mask multiply; one inline [128,1024] mask tile
serves every q-tile/head/batch. v is computed channel-major (per-partition
bias) and restriped kv-major by 4-tile DMA transposes. b1's QKV projection
runs as "pieces" interleaved into b0's ACT-paced attention chunks to fill PE
bubbles (drained before b0's A2A so its descriptors never queue behind the
collective).
"""

import numpy as np

B, S, D = 2, 2048, 1024
H = 16
HD = 64
NCORES = 8
QT = 256            # q-tile rows (also A2A shard rows per batch)
NQT = S // QT       # 8 q-tiles per batch
KVB = 128           # kv block size
ROWS = 512          # output rows per core (256 from each batch)

_CACHE = {}


def _build_nc():
    import ml_dtypes
    import concourse.bass as bass
    import concourse.bacc as bacc
    import concourse.mybir as mybir
    import concourse.tile as tile

    f32 = mybir.dt.float32
    bf16 = mybir.dt.bfloat16
    MULT = mybir.AluOpType.mult
    EXP = mybir.ActivationFunctionType.Exp

    nc = bacc.Bacc(num_devices=NCORES)

    xT_in = nc.dram_tensor("x_t", [128, B, 4, 8, 512], bf16, kind="ExternalInput")
    wq_in = nc.dram_tensor("w_q", [128, 8, 128], bf16, kind="ExternalInput")
    wk_in = nc.dram_tensor("w_k", [128, 8, 128], bf16, kind="ExternalInput")
    wv_in = nc.dram_tensor("w_v", [128, 8, 128], bf16, kind="ExternalInput")
    bq_in = nc.dram_tensor("b_q", [128, 1], f32, kind="ExternalInput")
    bk_in = nc.dram_tensor("b_k", [128, 1], f32, kind="ExternalInput")
    bv_in = nc.dram_tensor("b_v", [128, 1], f32, kind="ExternalInput")
    wo_in = nc.dram_tensor("w_o", [128, 8, D], bf16, kind="ExternalInput")
    y_out = nc.dram_tensor("y", [ROWS, D], f32, kind="ExternalOutput")

    # per-batch reshard: core c's output rows are batch-0 rows [256c, 256c+256)
    # plus batch-1 rows [256c, 256c+256), so each batch's AllToAll has uniform
    # junk-free 64KB shards and b0's A2A hides under b1's attention
    cc_in2 = [nc.dram_tensor(f"cc_in{b}", [NCORES, 128 * QT], bf16) for b in range(B)]
    cc_out2 = [nc.dram_tensor(f"cc_out{b}", [NCORES, 128 * QT], bf16) for b in range(B)]

    import os
    KDBG = bool(os.environ.get("KDBG"))
    if KDBG:
        dbg_q = nc.dram_tensor("dbg_q", [128, B, S], bf16, kind="ExternalOutput")
        dbg_k = nc.dram_tensor("dbg_k", [128, B, S], bf16, kind="ExternalOutput")
        dbg_v = nc.dram_tensor("dbg_v", [128, B, 16, 130], bf16, kind="ExternalOutput")
        dbg_ctx = nc.dram_tensor("dbg_ctx", [128, B, S], bf16, kind="ExternalOutput")
        dbg_den = nc.dram_tensor("dbg_den", [65, B, S], f32, kind="ExternalOutput")
        dbg_ca = nc.dram_tensor("dbg_ca", [128, 8, ROWS], bf16, kind="ExternalOutput")
        dbg_mask = nc.dram_tensor("dbg_mask", [128, 1024], bf16, kind="ExternalOutput")
        dbg_ats = nc.dram_tensor("dbg_ats", [128, 1024], bf16, kind="ExternalOutput")
        dbg_sc = nc.dram_tensor("dbg_sc", [128, 1024], f32, kind="ExternalOutput")

    # diagonal-block mask: ats segment layout [u0h0 | u1h0 | u0h1 | u1h1],
    # seg (h,u): valid iff u*128 + p <= r
    m_np = np.zeros((128, 1024), np.float32)
    pp = np.arange(128)[:, None]
    rr = np.arange(256)[None, :]
    for h in range(2):
        for u in range(2):
            m_np[:, (2 * h + u) * 256:(2 * h + u + 1) * 256] = (u * 128 + pp <= rr)
    mask_h = nc.inline_tensor(m_np.astype(ml_dtypes.bfloat16), name="mask_c")
    # den rows live at partitions 0 (h0) and 64 (h1) — engine writes must start
    # at 32-aligned partitions. E broadcasts those rows to the head halves.
    e_np = np.zeros((65, 128), np.float32)
    e_np[0, 0:64] = 1.0
    e_np[64, 64:128] = 1.0
    e2_h = nc.inline_tensor(e_np.astype(ml_dtypes.bfloat16), name="e2_c")

    # chunk schedule: strict global A/B alternation (A=2 jps, B=1 jp) so the
    # two PSUM score rings pipeline; jp t covers kv blocks (2t, 2t+1)
    chunks = []
    parity = 0
    for b in range(B):
        for j in range(NQT):
            rem, t = j + 1, 0
            while rem:
                n = min(2, rem) if parity == 0 else 1
                chunks.append(dict(b=b, j=j, t0=t, n=n, kind="AB"[parity],
                                   last=(rem - n == 0)))
                t += n
                rem -= n
                parity ^= 1
    nb0 = sum(1 for c in chunks if c["b"] == 0)

    with tile.TileContext(nc) as tc:
        with tc.tile_pool(name="const", bufs=1) as cpool, \
             tc.tile_pool(name="vsp", bufs=2) as vspool:
            # DMA order follows first use: the first qkv matmul needs only wk
            # and xT[b0, st0], not the full 8MB of xT
            wk = cpool.tile([128, 8, 128], bf16)
            nc.sync.dma_start(out=wk[:], in_=wk_in[:])
            bk = cpool.tile([128, 1], f32)
            nc.sync.dma_start(out=bk[:], in_=bk_in[:])
            xT = cpool.tile([128, B, 4, 8, 512], bf16)
            # st-major-contiguous host layout: 4KB contiguous per partition =
            # one cheap 2-D descriptor per 1MB stage slice
            nc.sync.dma_start(out=xT[:, 0, 0], in_=xT_in[:, 0, 0])
            wv = cpool.tile([128, 8, 128], bf16)
            nc.sync.dma_start(out=wv[:], in_=wv_in[:])
            bv = cpool.tile([128, 1], f32)
            nc.sync.dma_start(out=bv[:], in_=bv_in[:])
            wq = cpool.tile([128, 8, 128], bf16)
            nc.sync.dma_start(out=wq[:], in_=wq_in[:])
            bq = cpool.tile([128, 1], f32)
            nc.sync.dma_start(out=bq[:], in_=bq_in[:])
            mask = cpool.tile([128, 1024], bf16)
            nc.sync.dma_start(out=mask[:], in_=mask_h[:])
            e2 = cpool.tile([65, 128], bf16)
            nc.sync.dma_start(out=e2[:], in_=e2_h[:])
            for st in range(1, 4):
                nc.sync.dma_start(out=xT[:, 0, st], in_=xT_in[:, 0, st])
            for st in range(4):
                nc.sync.dma_start(out=xT[:, 1, st], in_=xT_in[:, 1, st])
            wo = cpool.tile([128, 8, D], bf16)
            nc.sync.dma_start(out=wo[:], in_=wo_in[:])

            kT = cpool.tile([128, B, S], bf16)
            qT = cpool.tile([128, B, S], bf16)
            vT = cpool.tile([128, B, S], bf16)
            vx = cpool.tile([128, B, 16, 130], bf16)
            ctxT = cpool.tile([128, B, S], bf16)
            ctx_all = cpool.tile([128, 8, ROWS], bf16)
            den_b = cpool.tile([65, B, S], f32)
            recf = cpool.tile([65, S], f32)
            rec_b = cpool.tile([65, B, S], bf16)

            # partitions 1..63 of den_b are never written; keep them at 1.0 so
            # the full-tile reciprocal stays finite (NaN*0 would poison bcast)
            nc.vector.memset(den_b[:], 1.0)
            # ones columns for the softmax denominator rows (slots 64, 129)
            nc.vector.memset(vx[:, :, :, 64:65], 1.0)
            nc.vector.memset(vx[:, :, :, 129:130], 1.0)

            def emit_qk_proj(b, st, wpan, bias, dest, pool, tag):
                ps = pool.tile([128, 512], f32, tag=tag, name=f"ps_{tag}_{b}_{st}")
                for db in range(8):
                    nc.tensor.matmul(ps[:], wpan[:, db, :], xT[:, b, st, db],
                                     start=(db == 0), stop=(db == 7))
                nc.vector.tensor_scalar_add(
                    dest[:, b, st * 512:(st + 1) * 512], ps[:], bias[:])

            def emit_v_proj(b, st, pool, tag):
                # channel-major v (bias is per-partition), then ONE DMA-transpose
                # per stage ([128,512] -> 4 transposed 128x128 tiles) and one
                # DVE copy into the kv-major v_ext layout
                emit_qk_proj(b, st, wv, bv, vT, pool, tag)
                vs4 = vspool.tile([128, 4, 128], bf16, tag="vs", name=f"vs_{b}_{st}")
                nc.sync.dma_start_transpose(
                    out=vs4[:], in_=vT[:, b, st * 512:(st + 1) * 512])
                nc.vector.tensor_copy(
                    out=vx[:, b, st * 4:(st + 1) * 4]
                        .rearrange("p t (h c) -> p t h c", h=2)[:, :, :, 0:64],
                    in_=vs4.rearrange("p t (h c) -> p t h c", h=2))

            # ---- Phase 1: batch-0 QKV projection ----
            with tc.tile_pool(name="p1ps", bufs=1, space="PSUM") as p1:
                for st in range(4):
                    emit_qk_proj(0, st, wk, bk, kT, p1, "qk0")
                    emit_v_proj(0, st, p1, "qk1")
                    emit_qk_proj(0, st, wq, bq, qT, p1, "qk0")

            # ---- Phase 2: attention (b0 + b1-qkv interleaved, then b1) ----
            with tc.tile_pool(name="p2ps", bufs=1, space="PSUM") as p2, \
                 tc.tile_pool(name="ats", bufs=2) as apool:

                # b1 qkv pieces fill PE bubbles of the ACT-paced attention.
                # Piece for stage st must be emitted before chunk (b1, j=2*st).
                # v pieces (which carry transpose DMAs) come first so they all
                # drain during b0 and never queue behind the in-flight b0
                # AllToAll; pure-matmul k/q pieces may spill into b1's early
                # chunks to fill its PE bubbles.
                def kp(st):
                    return (st, lambda st=st: emit_qk_proj(1, st, wk, bk, kT, p2, "pk"))

                def qp(st):
                    return (st, lambda st=st: emit_qk_proj(1, st, wq, bq, qT, p2, "pk"))

                def vp(st):
                    return (st, lambda st=st: emit_v_proj(1, st, p2, "pk"))

                pieces = [kp(0), vp(0), qp(0), vp(1), vp(2), vp(3),
                          kp(1), qp(1), kp(2), qp(2), kp(3), qp(3)]

                psc_cur = [None]

                def emit_chunk_qk(c, i):
                    b, j, t0, n = c["b"], c["j"], c["t0"], c["n"]
                    if c["kind"] == "A":
                        ps = p2.tile([128, 2048], f32, tag="scA", name=f"scA_{i}")
                    else:
                        ps = p2.tile([128, 1024], f32, tag="scB", name=f"scB_{i}")
                    ats = apool.tile([128, n * 1024], bf16, tag="at" + c["kind"],
                                     name=f"at_{i}")
                    for s in range(n):
                        t = t0 + s
                        for u in range(2):
                            kvb = 2 * t + u
                            for h in range(2):
                                nc.tensor.matmul(
                                    ps[:, s * 1024 + h * 512 + u * 256:
                                       s * 1024 + h * 512 + (u + 1) * 256],
                                    kT[h * 64:(h + 1) * 64, b, kvb * 128:(kvb + 1) * 128],
                                    qT[h * 64:(h + 1) * 64, b, j * QT:(j + 1) * QT],
                                    start=True, stop=True, tile_position=(h * 64, 0),
                                )
                    c["ps"] = ps
                    c["ats"] = ats

                def emit_chunk_tail(c):
                    b, j, t0, n = c["b"], c["j"], c["t0"], c["n"]
                    ps, ats = c["ps"], c["ats"]
                    nc.scalar.activation(ats[:], ps[:, 0:n * 1024], EXP)
                    if t0 + n - 1 == j:  # chunk contains the diagonal jp (t == j)
                        sd = j - t0
                        nc.vector.tensor_tensor(
                            out=ats[:, sd * 1024:(sd + 1) * 1024],
                            in0=ats[:, sd * 1024:(sd + 1) * 1024],
                            in1=mask[:], op=MULT)
                    if KDBG and b == 0 and j == 0:
                        nc.sync.dma_start(out=dbg_ats[:], in_=ats[:, 0:1024])
                        nc.sync.dma_start(out=dbg_mask[:], in_=mask[:])
                    if t0 == 0:
                        psc_cur[0] = p2.tile([65, 512], f32, tag="psc",
                                             name=f"psc_{b}_{j}")
                    psc = psc_cur[0]
                    for s in range(n):
                        t = t0 + s
                        for u in range(2):
                            kvb = 2 * t + u
                            for h in range(2):
                                # start=True on the FIRST PV only: it marks the
                                # whole 2KB bank pending-zero, so h1's first
                                # write overwrites its (stale) half and every
                                # later write accumulates — no zero-init matmul
                                # needed, and a per-h start would wipe h0.
                                nc.tensor.matmul(
                                    psc[:, h * 256:(h + 1) * 256],
                                    vx[:, b, kvb, h * 65:h * 65 + 65],
                                    ats[:, s * 1024 + h * 512 + u * 256:
                                        s * 1024 + h * 512 + (u + 1) * 256],
                                    start=(t == 0 and u == 0 and h == 0),
                                    stop=(t == j and u == 1),
                                    skip_group_check=True,
                                )
                    if c["last"]:
                        for h in range(2):
                            nc.vector.tensor_copy(
                                out=ctxT[h * 64:(h + 1) * 64, b, j * QT:(j + 1) * QT],
                                in_=psc[0:64, h * 256:(h + 1) * 256])
                            nc.vector.tensor_copy(
                                out=den_b[h * 64:h * 64 + 1, b, j * QT:(j + 1) * QT],
                                in_=psc[64:65, h * 256:(h + 1) * 256])
                        emit_norm_qtile(b, j)
                        if j == NQT - 1:
                            nc.gpsimd.collective_compute(
                                "AllToAll", mybir.AluOpType.bypass,
                                replica_groups=[list(range(NCORES))],
                                ins=[cc_in2[b][:]], outs=[cc_out2[b][:]],
                            )
                            for l in range(8):
                                nc.sync.dma_start(
                                    out=ctx_all[:, l, b * QT:(b + 1) * QT],
                                    in_=cc_out2[b][l, :].rearrange("(p s) -> p s", s=QT))

                def emit_norm_qtile(b, j):
                    sl = slice(j * QT, (j + 1) * QT)
                    # ~18 correct bits, 5x faster than reciprocal(); den>0 always
                    nc.vector.reciprocal_approx_fast(recf[:, sl], den_b[:, b, sl])
                    nc.vector.tensor_copy(out=rec_b[:, b, sl], in_=recf[:, sl])
                    bc = p2.tile([128, QT], f32, tag="pk", name=f"bc_{b}_{j}",
                                 padded_shape=[128, 512])
                    nc.tensor.matmul(bc[:], e2[:], rec_b[0:65, b, sl],
                                     start=True, stop=True)
                    nc.vector.tensor_tensor(
                        out=ctxT[:, b, sl], in0=ctxT[:, b, sl], in1=bc[:], op=MULT)
                    nc.sync.dma_start(
                        out=cc_in2[b][j, :].rearrange("(p s) -> p s", s=QT),
                        in_=ctxT[:, b, sl])

                # piece-slot plan: spread the 12 b1-qkv pieces over b0's chunks
                # and b1's early chunks, forced-flushing by deadline
                prev = None
                pieces_left = list(pieces)
                for i, c in enumerate(chunks):
                    if c["b"] == 1 and c["t0"] == 0:
                        due = [p for p in pieces_left if p[0] <= c["j"] // 2]
                        pieces_left = [p for p in pieces_left if p[0] > c["j"] // 2]
                        for p in due:
                            p[1]()
                    emit_chunk_qk(c, i)
                    if pieces_left and i % 2 == 0:
                        pieces_left.pop(0)[1]()
                    if prev is not None:
                        emit_chunk_tail(prev)
                    prev = c
                emit_chunk_tail(prev)

                # b0-row output projection overlaps the in-flight b1 AllToAll
                for rt in range(2):
                    for nh in range(2):
                        ps = p2.tile([128, 512], f32, tag="pk",
                                     name=f"py_{rt}_{nh}")
                        for cb in range(8):
                            nc.tensor.matmul(
                                ps[:], ctx_all[:, cb, rt * 128:(rt + 1) * 128],
                                wo[:, cb, nh * 512:(nh + 1) * 512],
                                start=(cb == 0), stop=(cb == 7))
                        yt = apool.tile([128, 512], f32, tag="yt",
                                        name=f"yt_{rt}_{nh}")
                        nc.vector.tensor_copy(out=yt[:], in_=ps[:])
                        nc.sync.dma_start(
                            out=y_out[rt * 128:(rt + 1) * 128,
                                      nh * 512:(nh + 1) * 512],
                            in_=yt[:])

            if KDBG:
                nc.sync.dma_start(out=dbg_q[:], in_=qT[:])
                nc.sync.dma_start(out=dbg_k[:], in_=kT[:])
                nc.sync.dma_start(out=dbg_v[:], in_=vx[:])
                nc.sync.dma_start(out=dbg_ctx[:], in_=ctxT[:])
                nc.sync.dma_start(out=dbg_den[:], in_=den_b[:])

            # ---- Phase 3: output projection on resharded ctx ----
            # ctx_all rows: [0:256) = batch-0 rows, [256:512) = batch-1 rows.
            # The b0 half (rt 0-1) only needs the b0 A2A, which finished during
            # b1's attention — it runs while the b1 A2A is still in flight.
            with tc.tile_pool(name="p3ps", bufs=2, space="PSUM") as p3, \
                 tc.tile_pool(name="p3sb", bufs=2) as p3sb:

                def outproj_rt(rt):
                    for nh in range(2):
                        ps = p3.tile([128, 512], f32, tag="y", name=f"py_{rt}_{nh}")
                        for cb in range(8):
                            nc.tensor.matmul(
                                ps[:], ctx_all[:, cb, rt * 128:(rt + 1) * 128],
                                wo[:, cb, nh * 512:(nh + 1) * 512],
                                start=(cb == 0), stop=(cb == 7))
                        yt = p3sb.tile([128, 512], f32, tag="yt", name=f"yt_{rt}_{nh}")
                        nc.vector.tensor_copy(out=yt[:], in_=ps[:])
                        nc.sync.dma_start(
                            out=y_out[rt * 128:(rt + 1) * 128, nh * 512:(nh + 1) * 512],
                            in_=yt[:])

                if KDBG:
                    nc.sync.dma_start(out=dbg_ca[:], in_=ctx_all[:])
                outproj_rt(2)
                outproj_rt(3)

    nc.finalize()
    return nc


def _host_inputs(x, W_qkv, b_qkv, W_out):
    import ml_dtypes

    x = np.asarray(x, np.float32)
    W_qkv = np.asarray(W_qkv, np.float32)
    b_qkv = np.asarray(b_qkv, np.float32)
    W_out = np.asarray(W_out, np.float32)

    # xT[p, b, st, db, s'] = x[b, st*512+s', db*128+p]
    xT = np.ascontiguousarray(
        x.transpose(2, 0, 1).reshape(8, 128, B, 4, 512).transpose(1, 2, 3, 0, 4)
    ).astype(ml_dtypes.bfloat16)
    wo_p = np.ascontiguousarray(
        W_out.reshape(8, 128, D).transpose(1, 0, 2)).astype(ml_dtypes.bfloat16)

    SCALE = 1.0 / np.sqrt(HD)
    in_maps = []
    for c in range(NCORES):
        co = 128 * c
        wq = (W_qkv[:, co:co + 128] * SCALE).reshape(8, 128, 128).transpose(1, 0, 2)
        wk = W_qkv[:, D + co:D + co + 128].reshape(8, 128, 128).transpose(1, 0, 2)
        wv = W_qkv[:, 2 * D + co:2 * D + co + 128].reshape(8, 128, 128).transpose(1, 0, 2)
        in_maps.append({
            "x_t": xT,
            "w_q": np.ascontiguousarray(wq).astype(ml_dtypes.bfloat16),
            "w_k": np.ascontiguousarray(wk).astype(ml_dtypes.bfloat16),
            "w_v": np.ascontiguousarray(wv).astype(ml_dtypes.bfloat16),
            "b_q": np.ascontiguousarray((b_qkv[co:co + 128] * SCALE).reshape(128, 1)),
            "b_k": np.ascontiguousarray(b_qkv[D + co:D + co + 128].reshape(128, 1)),
            "b_v": np.ascontiguousarray(b_qkv[2 * D + co:2 * D + co + 128].reshape(128, 1)),
            "w_o": wo_p,
        })
    return in_maps


def _run(in_maps, trace=False):
    from concourse.bass_utils import run_bass_kernel_spmd

    if "nc" not in _CACHE:
        _CACHE["nc"] = _build_nc()
    return run_bass_kernel_spmd(_CACHE["nc"], in_maps, core_ids=list(range(NCORES)),
                                trace=trace)


def _gather(res):
    out = np.empty((B, S, D), np.float32)
    for c in range(NCORES):
        y = res.results[c]["y"]
        out[0, c * QT:(c + 1) * QT, :] = y[0:QT]
        out[1, c * QT:(c + 1) * QT, :] = y[QT:2 * QT]
    return out


def kernel(x, W_qkv, b_qkv, W_out):
    in_maps = _host_inputs(x, W_qkv, b_qkv, W_out)
    res = _run(in_maps)
    return _gather(res)


# revision 60
# speedup vs baseline: 1.0872x; 1.0872x over previous
"""Causal multi-head attention (B=2, S=2048, D=1024, H=16) on 8 TRN2 NeuronCores.

Head-parallel (tensor-parallel) sharding: core c owns heads (2c, 2c+1) for
BOTH batches. The full x is fed to every core from host HBM (pre-transposed,
st-major-contiguous), so q/k/v for the core's heads are computed locally and
causal attention needs NO k/v collective at all (the v2 baseline spent ~170us
on 8 serialized AllGathers at ~45 GB/s mesh bandwidth).

Output rows are owned interleaved-by-batch (core c: batch-0 rows [256c,256c+256)
plus batch-1 rows [256c,256c+256)), so the head-split -> row-split ctx reshard
splits into TWO junk-free 512KB 8-way AllToAlls: batch 0's hides completely
under batch 1's attention; only batch 1's (~25us) is exposed, and the batch-0
half of the output projection runs while it is in flight.

Attention per (batch, 256-row q-tile j): kv blocks 0..2j+1 as job-pairs
(jp = 2 kv blocks x 2 heads), QK packed 2-heads-per-PE-pass via tile_position,
exp on ACT in alternating [128,2048]/[128,1024] chunks (A/B PSUM rings; ACT
cost is (N+352)/1.2GHz so bigger chunks amortize the fixed overhead), softmax
denominators via a ones-column in v (psc PSUM bank zero-initialized once per
q-tile because matmul start=True resets has_written for the whole 2KB bank),
per-q-tile normalization via reciprocal_approx_fast + E-matrix broadcast
matmul feeding the A2A staging as soon as each q-tile finishes. Only the
diagonal job-pair needs a mask multiply; one inline [128,1024] mask tile
serves every q-tile/head/batch. v is computed channel-major (per-partition
bias) and restriped kv-major by 4-tile DMA transposes. b1's QKV projection
runs as "pieces" interleaved into b0's ACT-paced attention chunks to fill PE
bubbles (drained before b0's A2A so its descriptors never queue behind the
collective).
"""

import numpy as np

B, S, D = 2, 2048, 1024
H = 16
HD = 64
NCORES = 8
QT = 256            # q-tile rows (also A2A shard rows per batch)
NQT = S // QT       # 8 q-tiles per batch
KVB = 128           # kv block size
ROWS = 512          # output rows per core (256 from each batch)

_CACHE = {}


def _build_nc():
    import ml_dtypes
    import concourse.bass as bass
    import concourse.bacc as bacc
    import concourse.mybir as mybir
    import concourse.tile as tile

    f32 = mybir.dt.float32
    bf16 = mybir.dt.bfloat16
    MULT = mybir.AluOpType.mult
    EXP = mybir.ActivationFunctionType.Exp

    nc = bacc.Bacc(num_devices=NCORES)

    xT_in = nc.dram_tensor("x_t", [128, B, 4, 8, 512], bf16, kind="ExternalInput")
    wq_in = nc.dram_tensor("w_q", [128, 8, 128], bf16, kind="ExternalInput")
    wk_in = nc.dram_tensor("w_k", [128, 8, 128], bf16, kind="ExternalInput")
    wv_in = nc.dram_tensor("w_v", [128, 8, 128], bf16, kind="ExternalInput")
    bq_in = nc.dram_tensor("b_q", [128, 1], f32, kind="ExternalInput")
    bk_in = nc.dram_tensor("b_k", [128, 1], f32, kind="ExternalInput")
    bv_in = nc.dram_tensor("b_v", [128, 1], f32, kind="ExternalInput")
    wo_in = nc.dram_tensor("w_o", [128, 8, D], bf16, kind="ExternalInput")
    y_out = nc.dram_tensor("y", [ROWS, D], f32, kind="ExternalOutput")

    # per-batch reshard: core c's output rows are batch-0 rows [256c, 256c+256)
    # plus batch-1 rows [256c, 256c+256), so each batch's AllToAll has uniform
    # junk-free 64KB shards and b0's A2A hides under b1's attention
    cc_in2 = [nc.dram_tensor(f"cc_in{b}", [NCORES, 128 * QT], bf16) for b in range(B)]
    cc_out2 = [nc.dram_tensor(f"cc_out{b}", [NCORES, 128 * QT], bf16) for b in range(B)]

    import os
    KDBG = bool(os.environ.get("KDBG"))
    if KDBG:
        dbg_q = nc.dram_tensor("dbg_q", [128, B, S], bf16, kind="ExternalOutput")
        dbg_k = nc.dram_tensor("dbg_k", [128, B, S], bf16, kind="ExternalOutput")
        dbg_v = nc.dram_tensor("dbg_v", [128, B, 16, 130], bf16, kind="ExternalOutput")
        dbg_ctx = nc.dram_tensor("dbg_ctx", [128, B, S], bf16, kind="ExternalOutput")
        dbg_den = nc.dram_tensor("dbg_den", [65, B, S], f32, kind="ExternalOutput")
        dbg_ca = nc.dram_tensor("dbg_ca", [128, 8, ROWS], bf16, kind="ExternalOutput")
        dbg_mask = nc.dram_tensor("dbg_mask", [128, 1024], bf16, kind="ExternalOutput")
        dbg_ats = nc.dram_tensor("dbg_ats", [128, 1024], bf16, kind="ExternalOutput")
        dbg_sc = nc.dram_tensor("dbg_sc", [128, 1024], f32, kind="ExternalOutput")

    # diagonal-block mask: ats segment layout [u0h0 | u1h0 | u0h1 | u1h1],
    # seg (h,u): valid iff u*128 + p <= r
    m_np = np.zeros((128, 1024), np.float32)
    pp = np.arange(128)[:, None]
    rr = np.arange(256)[None, :]
    for h in range(2):
        for u in range(2):
            m_np[:, (2 * h + u) * 256:(2 * h + u + 1) * 256] = (u * 128 + pp <= rr)
    mask_h = nc.inline_tensor(m_np.astype(ml_dtypes.bfloat16), name="mask_c")
    # den rows live at partitions 0 (h0) and 64 (h1) — engine writes must start
    # at 32-aligned partitions. E broadcasts those rows to the head halves.
    e_np = np.zeros((65, 128), np.float32)
    e_np[0, 0:64] = 1.0
    e_np[64, 64:128] = 1.0
    e2_h = nc.inline_tensor(e_np.astype(ml_dtypes.bfloat16), name="e2_c")

    # chunk schedule: strict global A/B alternation (A=2 jps, B=1 jp) so the
    # two PSUM score rings pipeline; jp t covers kv blocks (2t, 2t+1)
    chunks = []
    parity = 0
    for b in range(B):
        for j in range(NQT):
            rem, t = j + 1, 0
            while rem:
                n = min(2, rem) if parity == 0 else 1
                chunks.append(dict(b=b, j=j, t0=t, n=n, kind="AB"[parity],
                                   last=(rem - n == 0)))
                t += n
                rem -= n
                parity ^= 1
    nb0 = sum(1 for c in chunks if c["b"] == 0)

    with tile.TileContext(nc) as tc:
        with tc.tile_pool(name="const", bufs=1) as cpool, \
             tc.tile_pool(name="vsp", bufs=2) as vspool:
            # DMA order follows first use: the first qkv matmul needs only wk
            # and xT[b0, st0], not the full 8MB of xT
            wk = cpool.tile([128, 8, 128], bf16)
            nc.sync.dma_start(out=wk[:], in_=wk_in[:])
            bk = cpool.tile([128, 1], f32)
            nc.sync.dma_start(out=bk[:], in_=bk_in[:])
            xT = cpool.tile([128, B, 4, 8, 512], bf16)
            # st-major-contiguous host layout: 4KB contiguous per partition =
            # one cheap 2-D descriptor per 1MB stage slice
            nc.sync.dma_start(out=xT[:, 0, 0], in_=xT_in[:, 0, 0])
            wv = cpool.tile([128, 8, 128], bf16)
            nc.sync.dma_start(out=wv[:], in_=wv_in[:])
            bv = cpool.tile([128, 1], f32)
            nc.sync.dma_start(out=bv[:], in_=bv_in[:])
            wq = cpool.tile([128, 8, 128], bf16)
            nc.sync.dma_start(out=wq[:], in_=wq_in[:])
            bq = cpool.tile([128, 1], f32)
            nc.sync.dma_start(out=bq[:], in_=bq_in[:])
            mask = cpool.tile([128, 1024], bf16)
            nc.sync.dma_start(out=mask[:], in_=mask_h[:])
            e2 = cpool.tile([65, 128], bf16)
            nc.sync.dma_start(out=e2[:], in_=e2_h[:])
            for st in range(1, 4):
                nc.sync.dma_start(out=xT[:, 0, st], in_=xT_in[:, 0, st])
            for st in range(4):
                nc.sync.dma_start(out=xT[:, 1, st], in_=xT_in[:, 1, st])
            wo = cpool.tile([128, 8, D], bf16)
            nc.sync.dma_start(out=wo[:], in_=wo_in[:])

            kT = cpool.tile([128, B, S], bf16)
            qT = cpool.tile([128, B, S], bf16)
            vT = cpool.tile([128, B, S], bf16)
            vx = cpool.tile([128, B, 16, 130], bf16)
            ctxT = cpool.tile([128, B, S], bf16)
            ctx_all = cpool.tile([128, 8, ROWS], bf16)
            den_b = cpool.tile([65, B, S], f32)
            recf = cpool.tile([65, S], f32)
            rec_b = cpool.tile([65, B, S], bf16)

            # partitions 1..63 of den_b are never written; keep them at 1.0 so
            # the full-tile reciprocal stays finite (NaN*0 would poison bcast)
            nc.vector.memset(den_b[:], 1.0)
            # ones columns for the softmax denominator rows (slots 64, 129)
            nc.vector.memset(vx[:, :, :, 64:65], 1.0)
            nc.vector.memset(vx[:, :, :, 129:130], 1.0)

            def emit_qk_proj(b, st, wpan, bias, dest, pool, tag):
                ps = pool.tile([128, 512], f32, tag=tag, name=f"ps_{tag}_{b}_{st}")
                for db in range(8):
                    nc.tensor.matmul(ps[:], wpan[:, db, :], xT[:, b, st, db],
                                     start=(db == 0), stop=(db == 7))
                nc.vector.tensor_scalar_add(
                    dest[:, b, st * 512:(st + 1) * 512], ps[:], bias[:])

            def emit_v_proj(b, st, pool, tag):
                # channel-major v (bias is per-partition), then ONE DMA-transpose
                # per stage ([128,512] -> 4 transposed 128x128 tiles) and one
                # DVE copy into the kv-major v_ext layout
                emit_qk_proj(b, st, wv, bv, vT, pool, tag)
                vs4 = vspool.tile([128, 4, 128], bf16, tag="vs", name=f"vs_{b}_{st}")
                nc.sync.dma_start_transpose(
                    out=vs4[:], in_=vT[:, b, st * 512:(st + 1) * 512])
                nc.vector.tensor_copy(
                    out=vx[:, b, st * 4:(st + 1) * 4]
                        .rearrange("p t (h c) -> p t h c", h=2)[:, :, :, 0:64],
                    in_=vs4.rearrange("p t (h c) -> p t h c", h=2))

            # ---- Phase 1: batch-0 QKV projection ----
            with tc.tile_pool(name="p1ps", bufs=1, space="PSUM") as p1:
                for st in range(4):
                    emit_qk_proj(0, st, wk, bk, kT, p1, "qk0")
                    emit_v_proj(0, st, p1, "qk1")
                    emit_qk_proj(0, st, wq, bq, qT, p1, "qk0")

            # ---- Phase 2: attention (b0 + b1-qkv interleaved, then b1) ----
            with tc.tile_pool(name="p2ps", bufs=1, space="PSUM") as p2, \
                 tc.tile_pool(name="ats", bufs=2) as apool:

                # b1 qkv pieces fill PE bubbles of the ACT-paced attention.
                # Piece for stage st must be emitted before chunk (b1, j=2*st).
                # v pieces (which carry transpose DMAs) come first so they all
                # drain during b0 and never queue behind the in-flight b0
                # AllToAll; pure-matmul k/q pieces may spill into b1's early
                # chunks to fill its PE bubbles.
                def kp(st):
                    return (st, lambda st=st: emit_qk_proj(1, st, wk, bk, kT, p2, "pk"))

                def qp(st):
                    return (st, lambda st=st: emit_qk_proj(1, st, wq, bq, qT, p2, "pk"))

                def vp(st):
                    return (st, lambda st=st: emit_v_proj(1, st, p2, "pk"))

                pieces = [kp(0), vp(0), qp(0), kp(1), vp(1), qp(1),
                          kp(2), vp(2), qp(2), kp(3), vp(3), qp(3)]

                psc_cur = [None]

                def emit_chunk_qk(c, i):
                    b, j, t0, n = c["b"], c["j"], c["t0"], c["n"]
                    if c["kind"] == "A":
                        ps = p2.tile([128, 2048], f32, tag="scA", name=f"scA_{i}")
                    else:
                        ps = p2.tile([128, 1024], f32, tag="scB", name=f"scB_{i}")
                    ats = apool.tile([128, n * 1024], bf16, tag="at" + c["kind"],
                                     name=f"at_{i}")
                    for s in range(n):
                        t = t0 + s
                        for u in range(2):
                            kvb = 2 * t + u
                            for h in range(2):
                                nc.tensor.matmul(
                                    ps[:, s * 1024 + h * 512 + u * 256:
                                       s * 1024 + h * 512 + (u + 1) * 256],
                                    kT[h * 64:(h + 1) * 64, b, kvb * 128:(kvb + 1) * 128],
                                    qT[h * 64:(h + 1) * 64, b, j * QT:(j + 1) * QT],
                                    start=True, stop=True, tile_position=(h * 64, 0),
                                )
                    c["ps"] = ps
                    c["ats"] = ats

                def emit_chunk_tail(c):
                    b, j, t0, n = c["b"], c["j"], c["t0"], c["n"]
                    ps, ats = c["ps"], c["ats"]
                    nc.scalar.activation(ats[:], ps[:, 0:n * 1024], EXP)
                    if t0 + n - 1 == j:  # chunk contains the diagonal jp (t == j)
                        sd = j - t0
                        nc.vector.tensor_tensor(
                            out=ats[:, sd * 1024:(sd + 1) * 1024],
                            in0=ats[:, sd * 1024:(sd + 1) * 1024],
                            in1=mask[:], op=MULT)
                    if KDBG and b == 0 and j == 0:
                        nc.sync.dma_start(out=dbg_ats[:], in_=ats[:, 0:1024])
                        nc.sync.dma_start(out=dbg_mask[:], in_=mask[:])
                    if t0 == 0:
                        psc_cur[0] = p2.tile([65, 512], f32, tag="psc",
                                             name=f"psc_{b}_{j}")
                    psc = psc_cur[0]
                    for s in range(n):
                        t = t0 + s
                        for u in range(2):
                            kvb = 2 * t + u
                            for h in range(2):
                                # start=True on the FIRST PV only: it marks the
                                # whole 2KB bank pending-zero, so h1's first
                                # write overwrites its (stale) half and every
                                # later write accumulates — no zero-init matmul
                                # needed, and a per-h start would wipe h0.
                                nc.tensor.matmul(
                                    psc[:, h * 256:(h + 1) * 256],
                                    vx[:, b, kvb, h * 65:h * 65 + 65],
                                    ats[:, s * 1024 + h * 512 + u * 256:
                                        s * 1024 + h * 512 + (u + 1) * 256],
                                    start=(t == 0 and u == 0 and h == 0),
                                    stop=(t == j and u == 1),
                                    skip_group_check=True,
                                )
                    if c["last"]:
                        for h in range(2):
                            nc.vector.tensor_copy(
                                out=ctxT[h * 64:(h + 1) * 64, b, j * QT:(j + 1) * QT],
                                in_=psc[0:64, h * 256:(h + 1) * 256])
                            nc.vector.tensor_copy(
                                out=den_b[h * 64:h * 64 + 1, b, j * QT:(j + 1) * QT],
                                in_=psc[64:65, h * 256:(h + 1) * 256])
                        emit_norm_qtile(b, j)
                        if j == NQT - 1:
                            nc.gpsimd.collective_compute(
                                "AllToAll", mybir.AluOpType.bypass,
                                replica_groups=[list(range(NCORES))],
                                ins=[cc_in2[b][:]], outs=[cc_out2[b][:]],
                            )
                            for l in range(8):
                                nc.sync.dma_start(
                                    out=ctx_all[:, l, b * QT:(b + 1) * QT],
                                    in_=cc_out2[b][l, :].rearrange("(p s) -> p s", s=QT))

                def emit_norm_qtile(b, j):
                    sl = slice(j * QT, (j + 1) * QT)
                    # ~18 correct bits, 5x faster than reciprocal(); den>0 always
                    nc.vector.reciprocal_approx_fast(recf[:, sl], den_b[:, b, sl])
                    nc.vector.tensor_copy(out=rec_b[:, b, sl], in_=recf[:, sl])
                    bc = p2.tile([128, QT], f32, tag="pk", name=f"bc_{b}_{j}",
                                 padded_shape=[128, 512])
                    nc.tensor.matmul(bc[:], e2[:], rec_b[0:65, b, sl],
                                     start=True, stop=True)
                    nc.vector.tensor_tensor(
                        out=ctxT[:, b, sl], in0=ctxT[:, b, sl], in1=bc[:], op=MULT)
                    nc.sync.dma_start(
                        out=cc_in2[b][j, :].rearrange("(p s) -> p s", s=QT),
                        in_=ctxT[:, b, sl])

                # piece-slot plan: spread the 12 b1-qkv pieces over b0's chunks
                # and b1's early chunks, forced-flushing by deadline
                prev = None
                pieces_left = list(pieces)
                for i, c in enumerate(chunks):
                    if c["b"] == 1 and c["t0"] == 0:
                        due = [p for p in pieces_left if p[0] <= c["j"] // 2]
                        pieces_left = [p for p in pieces_left if p[0] > c["j"] // 2]
                        for p in due:
                            p[1]()
                    emit_chunk_qk(c, i)
                    if pieces_left and i % 2 == 0:
                        pieces_left.pop(0)[1]()
                    if prev is not None:
                        emit_chunk_tail(prev)
                    prev = c
                emit_chunk_tail(prev)

                # b0-row output projection overlaps the in-flight b1 AllToAll
                for rt in range(2):
                    for nh in range(2):
                        ps = p2.tile([128, 512], f32, tag="pk",
                                     name=f"py_{rt}_{nh}")
                        for cb in range(8):
                            nc.tensor.matmul(
                                ps[:], ctx_all[:, cb, rt * 128:(rt + 1) * 128],
                                wo[:, cb, nh * 512:(nh + 1) * 512],
                                start=(cb == 0), stop=(cb == 7))
                        yt = apool.tile([128, 512], f32, tag="yt",
                                        name=f"yt_{rt}_{nh}")
                        nc.vector.tensor_copy(out=yt[:], in_=ps[:])
                        nc.sync.dma_start(
                            out=y_out[rt * 128:(rt + 1) * 128,
                                      nh * 512:(nh + 1) * 512],
                            in_=yt[:])

            if KDBG:
                nc.sync.dma_start(out=dbg_q[:], in_=qT[:])
                nc.sync.dma_start(out=dbg_k[:], in_=kT[:])
                nc.sync.dma_start(out=dbg_v[:], in_=vx[:])
                nc.sync.dma_start(out=dbg_ctx[:], in_=ctxT[:])
                nc.sync.dma_start(out=dbg_den[:], in_=den_b[:])

            # ---- Phase 3: output projection on resharded ctx ----
            # ctx_all rows: [0:256) = batch-0 rows, [256:512) = batch-1 rows.
            # The b0 half (rt 0-1) only needs the b0 A2A, which finished during
            # b1's attention — it runs while the b1 A2A is still in flight.
            with tc.tile_pool(name="p3ps", bufs=2, space="PSUM") as p3, \
                 tc.tile_pool(name="p3sb", bufs=2) as p3sb:

                def outproj_rt(rt):
                    for nh in range(2):
                        ps = p3.tile([128, 512], f32, tag="y", name=f"py_{rt}_{nh}")
                        for cb in range(8):
                            nc.tensor.matmul(
                                ps[:], ctx_all[:, cb, rt * 128:(rt + 1) * 128],
                                wo[:, cb, nh * 512:(nh + 1) * 512],
                                start=(cb == 0), stop=(cb == 7))
                        yt = p3sb.tile([128, 512], f32, tag="yt", name=f"yt_{rt}_{nh}")
                        nc.vector.tensor_copy(out=yt[:], in_=ps[:])
                        nc.sync.dma_start(
                            out=y_out[rt * 128:(rt + 1) * 128, nh * 512:(nh + 1) * 512],
                            in_=yt[:])

                if KDBG:
                    nc.sync.dma_start(out=dbg_ca[:], in_=ctx_all[:])
                outproj_rt(2)
                outproj_rt(3)

    nc.finalize()
    return nc


def _host_inputs(x, W_qkv, b_qkv, W_out):
    import ml_dtypes

    x = np.asarray(x, np.float32)
    W_qkv = np.asarray(W_qkv, np.float32)
    b_qkv = np.asarray(b_qkv, np.float32)
    W_out = np.asarray(W_out, np.float32)

    # xT[p, b, st, db, s'] = x[b, st*512+s', db*128+p]
    xT = np.ascontiguousarray(
        x.transpose(2, 0, 1).reshape(8, 128, B, 4, 512).transpose(1, 2, 3, 0, 4)
    ).astype(ml_dtypes.bfloat16)
    wo_p = np.ascontiguousarray(
        W_out.reshape(8, 128, D).transpose(1, 0, 2)).astype(ml_dtypes.bfloat16)

    SCALE = 1.0 / np.sqrt(HD)
    in_maps = []
    for c in range(NCORES):
        co = 128 * c
        wq = (W_qkv[:, co:co + 128] * SCALE).reshape(8, 128, 128).transpose(1, 0, 2)
        wk = W_qkv[:, D + co:D + co + 128].reshape(8, 128, 128).transpose(1, 0, 2)
        wv = W_qkv[:, 2 * D + co:2 * D + co + 128].reshape(8, 128, 128).transpose(1, 0, 2)
        in_maps.append({
            "x_t": xT,
            "w_q": np.ascontiguousarray(wq).astype(ml_dtypes.bfloat16),
            "w_k": np.ascontiguousarray(wk).astype(ml_dtypes.bfloat16),
            "w_v": np.ascontiguousarray(wv).astype(ml_dtypes.bfloat16),
            "b_q": np.ascontiguousarray((b_qkv[co:co + 128] * SCALE).reshape(128, 1)),
            "b_k": np.ascontiguousarray(b_qkv[D + co:D + co + 128].reshape(128, 1)),
            "b_v": np.ascontiguousarray(b_qkv[2 * D + co:2 * D + co + 128].reshape(128, 1)),
            "w_o": wo_p,
        })
    return in_maps


def _run(in_maps, trace=False):
    from concourse.bass_utils import run_bass_kernel_spmd

    if "nc" not in _CACHE:
        _CACHE["nc"] = _build_nc()
    return run_bass_kernel_spmd(_CACHE["nc"], in_maps, core_ids=list(range(NCORES)),
                                trace=trace)


def _gather(res):
    out = np.empty((B, S, D), np.float32)
    for c in range(NCORES):
        y = res.results[c]["y"]
        out[0, c * QT:(c + 1) * QT, :] = y[0:QT]
        out[1, c * QT:(c + 1) * QT, :] = y[QT:2 * QT]
    return out


def kernel(x, W_qkv, b_qkv, W_out):
    in_maps = _host_inputs(x, W_qkv, b_qkv, W_out)
    res = _run(in_maps)
    return _gather(res)


# revision 62
# speedup vs baseline: 1.0972x; 1.0091x over previous
"""Causal multi-head attention (B=2, S=2048, D=1024, H=16) on 8 TRN2 NeuronCores.

Head-parallel (tensor-parallel) sharding: core c owns heads (2c, 2c+1) for
BOTH batches. The full x is fed to every core from host HBM (pre-transposed,
st-major-contiguous), so q/k/v for the core's heads are computed locally and
causal attention needs NO k/v collective at all (the v2 baseline spent ~170us
on 8 serialized AllGathers at ~45 GB/s mesh bandwidth).

Output rows are owned interleaved-by-batch (core c: batch-0 rows [256c,256c+256)
plus batch-1 rows [256c,256c+256)), so the head-split -> row-split ctx reshard
splits into TWO junk-free 512KB 8-way AllToAlls: batch 0's hides completely
under batch 1's attention; only batch 1's (~25us) is exposed, and the batch-0
half of the output projection runs while it is in flight.

Attention per (batch, 256-row q-tile j): kv blocks 0..2j+1 as job-pairs
(jp = 2 kv blocks x 2 heads), QK packed 2-heads-per-PE-pass via tile_position,
exp on ACT in alternating [128,2048]/[128,1024] chunks (A/B PSUM rings; ACT
cost is (N+352)/1.2GHz so bigger chunks amortize the fixed overhead), softmax
denominators via a ones-column in v (psc PSUM bank zero-initialized once per
q-tile because matmul start=True resets has_written for the whole 2KB bank),
per-q-tile normalization via reciprocal_approx_fast + E-matrix broadcast
matmul feeding the A2A staging as soon as each q-tile finishes. Only the
diagonal job-pair needs a mask multiply; one inline [128,1024] mask tile
serves every q-tile/head/batch. v is computed channel-major (per-partition
bias) and restriped kv-major by 4-tile DMA transposes. b1's QKV projection
runs as "pieces" interleaved into b0's ACT-paced attention chunks to fill PE
bubbles (drained before b0's A2A so its descriptors never queue behind the
collective).
"""

import numpy as np

B, S, D = 2, 2048, 1024
H = 16
HD = 64
NCORES = 8
QT = 256            # q-tile rows (also A2A shard rows per batch)
NQT = S // QT       # 8 q-tiles per batch
KVB = 128           # kv block size
ROWS = 512          # output rows per core (256 from each batch)

_CACHE = {}


def _build_nc():
    import ml_dtypes
    import concourse.bass as bass
    import concourse.bacc as bacc
    import concourse.mybir as mybir
    import concourse.tile as tile

    f32 = mybir.dt.float32
    bf16 = mybir.dt.bfloat16
    MULT = mybir.AluOpType.mult
    EXP = mybir.ActivationFunctionType.Exp

    nc = bacc.Bacc(num_devices=NCORES)

    xT_in = nc.dram_tensor("x_t", [128, B, 4, 8, 512], bf16, kind="ExternalInput")
    wq_in = nc.dram_tensor("w_q", [128, 8, 128], bf16, kind="ExternalInput")
    wk_in = nc.dram_tensor("w_k", [128, 8, 128], bf16, kind="ExternalInput")
    wv_in = nc.dram_tensor("w_v", [128, 8, 128], bf16, kind="ExternalInput")
    bq_in = nc.dram_tensor("b_q", [128, 1], f32, kind="ExternalInput")
    bk_in = nc.dram_tensor("b_k", [128, 1], f32, kind="ExternalInput")
    bv_in = nc.dram_tensor("b_v", [128, 1], f32, kind="ExternalInput")
    wo_in = nc.dram_tensor("w_o", [128, 8, D], bf16, kind="ExternalInput")
    y_out = nc.dram_tensor("y", [ROWS, D], f32, kind="ExternalOutput")

    # per-batch reshard: core c's output rows are batch-0 rows [256c, 256c+256)
    # plus batch-1 rows [256c, 256c+256), so each batch's AllToAll has uniform
    # junk-free 64KB shards and b0's A2A hides under b1's attention
    cc_in2 = [nc.dram_tensor(f"cc_in{b}", [NCORES, 128 * QT], bf16) for b in range(B)]
    cc_out2 = [nc.dram_tensor(f"cc_out{b}", [NCORES, 128 * QT], bf16) for b in range(B)]

    import os
    KDBG = bool(os.environ.get("KDBG"))
    if KDBG:
        dbg_q = nc.dram_tensor("dbg_q", [128, B, S], bf16, kind="ExternalOutput")
        dbg_k = nc.dram_tensor("dbg_k", [128, B, S], bf16, kind="ExternalOutput")
        dbg_v = nc.dram_tensor("dbg_v", [128, B, 16, 130], bf16, kind="ExternalOutput")
        dbg_ctx = nc.dram_tensor("dbg_ctx", [128, B, S], bf16, kind="ExternalOutput")
        dbg_den = nc.dram_tensor("dbg_den", [65, B, S], f32, kind="ExternalOutput")
        dbg_ca = nc.dram_tensor("dbg_ca", [128, 8, ROWS], bf16, kind="ExternalOutput")
        dbg_mask = nc.dram_tensor("dbg_mask", [128, 1024], bf16, kind="ExternalOutput")
        dbg_ats = nc.dram_tensor("dbg_ats", [128, 1024], bf16, kind="ExternalOutput")
        dbg_sc = nc.dram_tensor("dbg_sc", [128, 1024], f32, kind="ExternalOutput")

    # diagonal-block mask: ats segment layout [u0h0 | u1h0 | u0h1 | u1h1],
    # seg (h,u): valid iff u*128 + p <= r
    m_np = np.zeros((128, 1024), np.float32)
    pp = np.arange(128)[:, None]
    rr = np.arange(256)[None, :]
    for h in range(2):
        for u in range(2):
            m_np[:, (2 * h + u) * 256:(2 * h + u + 1) * 256] = (u * 128 + pp <= rr)
    mask_h = nc.inline_tensor(m_np.astype(ml_dtypes.bfloat16), name="mask_c")
    # den rows live at partitions 0 (h0) and 64 (h1) — engine writes must start
    # at 32-aligned partitions. E broadcasts those rows to the head halves.
    e_np = np.zeros((65, 128), np.float32)
    e_np[0, 0:64] = 1.0
    e_np[64, 64:128] = 1.0
    e2_h = nc.inline_tensor(e_np.astype(ml_dtypes.bfloat16), name="e2_c")

    # chunk schedule: strict global A/B alternation (A=2 jps, B=1 jp) so the
    # two PSUM score rings pipeline; jp t covers kv blocks (2t, 2t+1).
    # q-tiles run largest-first: big chunks prime the exp/PV pipeline and the
    # final norm -> A2A trigger chain sits behind a small q-tile.
    chunks = []
    parity = 0
    for b in range(B):
        for j in reversed(range(NQT)):
            rem, t = j + 1, 0
            while rem:
                n = min(2, rem) if parity == 0 else 1
                chunks.append(dict(b=b, j=j, t0=t, n=n, kind="AB"[parity],
                                   last=(rem - n == 0)))
                t += n
                rem -= n
                parity ^= 1
    nb0 = sum(1 for c in chunks if c["b"] == 0)

    with tile.TileContext(nc) as tc:
        with tc.tile_pool(name="const", bufs=1) as cpool, \
             tc.tile_pool(name="vsp", bufs=2) as vspool:
            # DMA order follows first use: the first qkv matmul needs only wk
            # and xT[b0, st0], not the full 8MB of xT
            wk = cpool.tile([128, 8, 128], bf16)
            nc.sync.dma_start(out=wk[:], in_=wk_in[:])
            bk = cpool.tile([128, 1], f32)
            nc.sync.dma_start(out=bk[:], in_=bk_in[:])
            xT = cpool.tile([128, B, 4, 8, 512], bf16)
            # st-major-contiguous host layout: 4KB contiguous per partition =
            # one cheap 2-D descriptor per 1MB stage slice
            nc.sync.dma_start(out=xT[:, 0, 0], in_=xT_in[:, 0, 0])
            wv = cpool.tile([128, 8, 128], bf16)
            nc.sync.dma_start(out=wv[:], in_=wv_in[:])
            bv = cpool.tile([128, 1], f32)
            nc.sync.dma_start(out=bv[:], in_=bv_in[:])
            wq = cpool.tile([128, 8, 128], bf16)
            nc.sync.dma_start(out=wq[:], in_=wq_in[:])
            bq = cpool.tile([128, 1], f32)
            nc.sync.dma_start(out=bq[:], in_=bq_in[:])
            mask = cpool.tile([128, 1024], bf16)
            nc.sync.dma_start(out=mask[:], in_=mask_h[:])
            e2 = cpool.tile([65, 128], bf16)
            nc.sync.dma_start(out=e2[:], in_=e2_h[:])
            for st in range(1, 4):
                nc.sync.dma_start(out=xT[:, 0, st], in_=xT_in[:, 0, st])
            for st in range(4):
                nc.sync.dma_start(out=xT[:, 1, st], in_=xT_in[:, 1, st])
            wo = cpool.tile([128, 8, D], bf16)
            nc.sync.dma_start(out=wo[:], in_=wo_in[:])

            kT = cpool.tile([128, B, S], bf16)
            qT = cpool.tile([128, B, S], bf16)
            vT = cpool.tile([128, B, S], bf16)
            vx = cpool.tile([128, B, 16, 130], bf16)
            ctxT = cpool.tile([128, B, S], bf16)
            ctx_all = cpool.tile([128, 8, ROWS], bf16)
            den_b = cpool.tile([65, B, S], f32)
            recf = cpool.tile([65, S], f32)
            rec_b = cpool.tile([65, B, S], bf16)

            # partitions 1..63 of den_b are never written; keep them at 1.0 so
            # the full-tile reciprocal stays finite (NaN*0 would poison bcast)
            nc.vector.memset(den_b[:], 1.0)
            # ones columns for the softmax denominator rows (slots 64, 129)
            nc.vector.memset(vx[:, :, :, 64:65], 1.0)
            nc.vector.memset(vx[:, :, :, 129:130], 1.0)

            def emit_qk_proj(b, st, wpan, bias, dest, pool, tag):
                ps = pool.tile([128, 512], f32, tag=tag, name=f"ps_{tag}_{b}_{st}")
                for db in range(8):
                    nc.tensor.matmul(ps[:], wpan[:, db, :], xT[:, b, st, db],
                                     start=(db == 0), stop=(db == 7))
                nc.vector.tensor_scalar_add(
                    dest[:, b, st * 512:(st + 1) * 512], ps[:], bias[:])

            def emit_v_proj(b, st, pool, tag):
                # channel-major v (bias is per-partition), then ONE DMA-transpose
                # per stage ([128,512] -> 4 transposed 128x128 tiles) and one
                # DVE copy into the kv-major v_ext layout
                emit_qk_proj(b, st, wv, bv, vT, pool, tag)
                vs4 = vspool.tile([128, 4, 128], bf16, tag="vs", name=f"vs_{b}_{st}")
                nc.sync.dma_start_transpose(
                    out=vs4[:], in_=vT[:, b, st * 512:(st + 1) * 512])
                nc.vector.tensor_copy(
                    out=vx[:, b, st * 4:(st + 1) * 4]
                        .rearrange("p t (h c) -> p t h c", h=2)[:, :, :, 0:64],
                    in_=vs4.rearrange("p t (h c) -> p t h c", h=2))

            # ---- Phase 1: batch-0 QKV projection ----
            with tc.tile_pool(name="p1ps", bufs=1, space="PSUM") as p1:
                for st in range(4):
                    emit_qk_proj(0, st, wk, bk, kT, p1, "qk0")
                    emit_v_proj(0, st, p1, "qk1")
                    emit_qk_proj(0, st, wq, bq, qT, p1, "qk0")

            # ---- Phase 2: attention (b0 + b1-qkv interleaved, then b1) ----
            with tc.tile_pool(name="p2ps", bufs=1, space="PSUM") as p2, \
                 tc.tile_pool(name="ats", bufs=2) as apool:

                # b1 qkv pieces fill PE bubbles of the ACT-paced attention.
                # Piece for stage st must be emitted before chunk (b1, j=2*st).
                # v pieces (which carry transpose DMAs) come first so they all
                # drain during b0 and never queue behind the in-flight b0
                # AllToAll; pure-matmul k/q pieces may spill into b1's early
                # chunks to fill its PE bubbles.
                def kp(st):
                    return (st, lambda st=st: emit_qk_proj(1, st, wk, bk, kT, p2, "pk"))

                def qp(st):
                    return (st, lambda st=st: emit_qk_proj(1, st, wq, bq, qT, p2, "pk"))

                def vp(st):
                    return (st, lambda st=st: emit_v_proj(1, st, p2, "pk"))

                pieces = [kp(0), vp(0), qp(0), kp(1), vp(1), qp(1),
                          kp(2), vp(2), qp(2), kp(3), vp(3), qp(3)]

                psc_cur = [None]

                def emit_chunk_qk(c, i):
                    b, j, t0, n = c["b"], c["j"], c["t0"], c["n"]
                    if c["kind"] == "A":
                        ps = p2.tile([128, 2048], f32, tag="scA", name=f"scA_{i}")
                    else:
                        ps = p2.tile([128, 1024], f32, tag="scB", name=f"scB_{i}")
                    ats = apool.tile([128, n * 1024], bf16, tag="at" + c["kind"],
                                     name=f"at_{i}")
                    for s in range(n):
                        t = t0 + s
                        for u in range(2):
                            kvb = 2 * t + u
                            for h in range(2):
                                nc.tensor.matmul(
                                    ps[:, s * 1024 + h * 512 + u * 256:
                                       s * 1024 + h * 512 + (u + 1) * 256],
                                    kT[h * 64:(h + 1) * 64, b, kvb * 128:(kvb + 1) * 128],
                                    qT[h * 64:(h + 1) * 64, b, j * QT:(j + 1) * QT],
                                    start=True, stop=True, tile_position=(h * 64, 0),
                                )
                    c["ps"] = ps
                    c["ats"] = ats

                def emit_chunk_tail(c):
                    b, j, t0, n = c["b"], c["j"], c["t0"], c["n"]
                    ps, ats = c["ps"], c["ats"]
                    nc.scalar.activation(ats[:], ps[:, 0:n * 1024], EXP)
                    if t0 + n - 1 == j:  # chunk contains the diagonal jp (t == j)
                        sd = j - t0
                        nc.vector.tensor_tensor(
                            out=ats[:, sd * 1024:(sd + 1) * 1024],
                            in0=ats[:, sd * 1024:(sd + 1) * 1024],
                            in1=mask[:], op=MULT)
                    if KDBG and b == 0 and j == 0:
                        nc.sync.dma_start(out=dbg_ats[:], in_=ats[:, 0:1024])
                        nc.sync.dma_start(out=dbg_mask[:], in_=mask[:])
                    if t0 == 0:
                        psc_cur[0] = p2.tile([65, 512], f32, tag="psc",
                                             name=f"psc_{b}_{j}")
                    psc = psc_cur[0]
                    for s in range(n):
                        t = t0 + s
                        for u in range(2):
                            kvb = 2 * t + u
                            for h in range(2):
                                # start=True on the FIRST PV only: it marks the
                                # whole 2KB bank pending-zero, so h1's first
                                # write overwrites its (stale) half and every
                                # later write accumulates — no zero-init matmul
                                # needed, and a per-h start would wipe h0.
                                nc.tensor.matmul(
                                    psc[:, h * 256:(h + 1) * 256],
                                    vx[:, b, kvb, h * 65:h * 65 + 65],
                                    ats[:, s * 1024 + h * 512 + u * 256:
                                        s * 1024 + h * 512 + (u + 1) * 256],
                                    start=(t == 0 and u == 0 and h == 0),
                                    stop=(t == j and u == 1),
                                    skip_group_check=True,
                                )
                    if c["last"]:
                        for h in range(2):
                            nc.vector.tensor_copy(
                                out=ctxT[h * 64:(h + 1) * 64, b, j * QT:(j + 1) * QT],
                                in_=psc[0:64, h * 256:(h + 1) * 256])
                            nc.vector.tensor_copy(
                                out=den_b[h * 64:h * 64 + 1, b, j * QT:(j + 1) * QT],
                                in_=psc[64:65, h * 256:(h + 1) * 256])
                        emit_norm_qtile(b, j)
                        if j == 0:  # q-tiles run largest-first; j=0 is last
                            nc.gpsimd.collective_compute(
                                "AllToAll", mybir.AluOpType.bypass,
                                replica_groups=[list(range(NCORES))],
                                ins=[cc_in2[b][:]], outs=[cc_out2[b][:]],
                            )
                            for l in range(8):
                                nc.sync.dma_start(
                                    out=ctx_all[:, l, b * QT:(b + 1) * QT],
                                    in_=cc_out2[b][l, :].rearrange("(p s) -> p s", s=QT))

                def emit_norm_qtile(b, j):
                    sl = slice(j * QT, (j + 1) * QT)
                    # ~18 correct bits, 5x faster than reciprocal(); den>0 always
                    nc.vector.reciprocal_approx_fast(recf[:, sl], den_b[:, b, sl])
                    nc.vector.tensor_copy(out=rec_b[:, b, sl], in_=recf[:, sl])
                    bc = p2.tile([128, QT], f32, tag="pk", name=f"bc_{b}_{j}",
                                 padded_shape=[128, 512])
                    nc.tensor.matmul(bc[:], e2[:], rec_b[0:65, b, sl],
                                     start=True, stop=True)
                    nc.vector.tensor_tensor(
                        out=ctxT[:, b, sl], in0=ctxT[:, b, sl], in1=bc[:], op=MULT)
                    nc.sync.dma_start(
                        out=cc_in2[b][j, :].rearrange("(p s) -> p s", s=QT),
                        in_=ctxT[:, b, sl])

                # piece-slot plan: spread the 12 b1-qkv pieces over b0's chunks
                # and b1's early chunks, forced-flushing by deadline
                prev = None
                pieces_left = list(pieces)
                for i, c in enumerate(chunks):
                    if c["b"] == 1 and c["t0"] == 0:
                        due = [p for p in pieces_left if p[0] <= c["j"] // 2]
                        pieces_left = [p for p in pieces_left if p[0] > c["j"] // 2]
                        for p in due:
                            p[1]()
                    emit_chunk_qk(c, i)
                    if pieces_left and i % 2 == 0:
                        pieces_left.pop(0)[1]()
                    if prev is not None:
                        emit_chunk_tail(prev)
                    prev = c
                emit_chunk_tail(prev)

                # b0-row output projection overlaps the in-flight b1 AllToAll
                for rt in range(2):
                    for nh in range(2):
                        ps = p2.tile([128, 512], f32, tag="pk",
                                     name=f"py_{rt}_{nh}")
                        for cb in range(8):
                            nc.tensor.matmul(
                                ps[:], ctx_all[:, cb, rt * 128:(rt + 1) * 128],
                                wo[:, cb, nh * 512:(nh + 1) * 512],
                                start=(cb == 0), stop=(cb == 7))
                        yt = apool.tile([128, 512], f32, tag="yt",
                                        name=f"yt_{rt}_{nh}")
                        nc.vector.tensor_copy(out=yt[:], in_=ps[:])
                        nc.sync.dma_start(
                            out=y_out[rt * 128:(rt + 1) * 128,
                                      nh * 512:(nh + 1) * 512],
                            in_=yt[:])

            if KDBG:
                nc.sync.dma_start(out=dbg_q[:], in_=qT[:])
                nc.sync.dma_start(out=dbg_k[:], in_=kT[:])
                nc.sync.dma_start(out=dbg_v[:], in_=vx[:])
                nc.sync.dma_start(out=dbg_ctx[:], in_=ctxT[:])
                nc.sync.dma_start(out=dbg_den[:], in_=den_b[:])

            # ---- Phase 3: output projection on resharded ctx ----
            # ctx_all rows: [0:256) = batch-0 rows, [256:512) = batch-1 rows.
            # The b0 half (rt 0-1) only needs the b0 A2A, which finished during
            # b1's attention — it runs while the b1 A2A is still in flight.
            with tc.tile_pool(name="p3ps", bufs=2, space="PSUM") as p3, \
                 tc.tile_pool(name="p3sb", bufs=2) as p3sb:

                def outproj_rt(rt):
                    for nh in range(2):
                        ps = p3.tile([128, 512], f32, tag="y", name=f"py_{rt}_{nh}")
                        for cb in range(8):
                            nc.tensor.matmul(
                                ps[:], ctx_all[:, cb, rt * 128:(rt + 1) * 128],
                                wo[:, cb, nh * 512:(nh + 1) * 512],
                                start=(cb == 0), stop=(cb == 7))
                        yt = p3sb.tile([128, 512], f32, tag="yt", name=f"yt_{rt}_{nh}")
                        nc.vector.tensor_copy(out=yt[:], in_=ps[:])
                        nc.sync.dma_start(
                            out=y_out[rt * 128:(rt + 1) * 128, nh * 512:(nh + 1) * 512],
                            in_=yt[:])

                if KDBG:
                    nc.sync.dma_start(out=dbg_ca[:], in_=ctx_all[:])
                outproj_rt(2)
                outproj_rt(3)

    nc.finalize()
    return nc


def _host_inputs(x, W_qkv, b_qkv, W_out):
    import ml_dtypes

    x = np.asarray(x, np.float32)
    W_qkv = np.asarray(W_qkv, np.float32)
    b_qkv = np.asarray(b_qkv, np.float32)
    W_out = np.asarray(W_out, np.float32)

    # xT[p, b, st, db, s'] = x[b, st*512+s', db*128+p]
    xT = np.ascontiguousarray(
        x.transpose(2, 0, 1).reshape(8, 128, B, 4, 512).transpose(1, 2, 3, 0, 4)
    ).astype(ml_dtypes.bfloat16)
    wo_p = np.ascontiguousarray(
        W_out.reshape(8, 128, D).transpose(1, 0, 2)).astype(ml_dtypes.bfloat16)

    SCALE = 1.0 / np.sqrt(HD)
    in_maps = []
    for c in range(NCORES):
        co = 128 * c
        wq = (W_qkv[:, co:co + 128] * SCALE).reshape(8, 128, 128).transpose(1, 0, 2)
        wk = W_qkv[:, D + co:D + co + 128].reshape(8, 128, 128).transpose(1, 0, 2)
        wv = W_qkv[:, 2 * D + co:2 * D + co + 128].reshape(8, 128, 128).transpose(1, 0, 2)
        in_maps.append({
            "x_t": xT,
            "w_q": np.ascontiguousarray(wq).astype(ml_dtypes.bfloat16),
            "w_k": np.ascontiguousarray(wk).astype(ml_dtypes.bfloat16),
            "w_v": np.ascontiguousarray(wv).astype(ml_dtypes.bfloat16),
            "b_q": np.ascontiguousarray((b_qkv[co:co + 128] * SCALE).reshape(128, 1)),
            "b_k": np.ascontiguousarray(b_qkv[D + co:D + co + 128].reshape(128, 1)),
            "b_v": np.ascontiguousarray(b_qkv[2 * D + co:2 * D + co + 128].reshape(128, 1)),
            "w_o": wo_p,
        })
    return in_maps


def _run(in_maps, trace=False):
    from concourse.bass_utils import run_bass_kernel_spmd

    if "nc" not in _CACHE:
        _CACHE["nc"] = _build_nc()
    return run_bass_kernel_spmd(_CACHE["nc"], in_maps, core_ids=list(range(NCORES)),
                                trace=trace)


def _gather(res):
    out = np.empty((B, S, D), np.float32)
    for c in range(NCORES):
        y = res.results[c]["y"]
        out[0, c * QT:(c + 1) * QT, :] = y[0:QT]
        out[1, c * QT:(c + 1) * QT, :] = y[QT:2 * QT]
    return out


def kernel(x, W_qkv, b_qkv, W_out):
    in_maps = _host_inputs(x, W_qkv, b_qkv, W_out)
    res = _run(in_maps)
    return _gather(res)
